# revision 2
# baseline (speedup 1.0000x reference)
"""LRU (linear recurrent unit) kernel for Trainium2, 8 NeuronCores.

Math (matching the reference):
    s   = x @ w_in.T + b_in                      # (n, b, 2d)
    u   = gamma * (s interleaved-split to complex)  # u = ur + i*ui, (n, b, d)
    h_t = a * h_{t-1} + u_t,  a = nu * e^{i*theta}  (per-channel constant)
    out = [Re h, Im h] @ w_out.T + b_out

Device strategy (per core, batch-parallel over 8 cores):
    - in_proj / out_proj on PE in fp16 (fp32 PSUM accumulation); gamma and
      the (e k) de-interleave permutation are folded into w_in on the host.
    - The complex scan is diagonalized with the polar trick: with
      m_t = e^{-i*theta*t} h_t the recurrence becomes  m_t = nu*m_{t-1} + u'_t
      with REAL coefficient nu, where u'_t = e^{-i*theta*t} u_t.  The two real
      scans run on the native tensor_tensor_scan instruction (fp32 state).
      cos/sin rotation tables are precomputed on the host.
    - Streaming over sequence chunks; scan carry chained via the last column
      of the previous chunk's scan output.
"""

import numpy as np

import concourse.bass as bass
import concourse.tile as tile
from concourse import bacc, mybir
from concourse import bass_utils

N, B, D = 2048, 8, 1024
P = 128
CHUNK = 512
NCHUNKS = N // CHUNK        # 4
DT = D // P                 # 8 d-tiles of 128 channels
ET = 2 * DT                 # 16 e'-tiles (real block then imag block)
NCORES = 8

F16 = mybir.dt.float16
F32 = mybir.dt.float32
MULT = mybir.AluOpType.mult
ADD = mybir.AluOpType.add


def build_program(n_len: int = N, chunk: int = CHUNK):
    nchunks = n_len // chunk
    assert n_len % chunk == 0 and chunk % P == 0 and D % 512 == 0
    nc = bacc.Bacc(
        "TRN2", target_bir_lowering=False, debug=False, num_devices=NCORES
    )

    xT = nc.dram_tensor("xT", [nchunks, P, DT, chunk], F16, kind="ExternalInput").ap()
    ct = nc.dram_tensor("ct", [nchunks, P, DT, chunk], F16, kind="ExternalInput").ap()
    st = nc.dram_tensor("st", [nchunks, P, DT, chunk], F16, kind="ExternalInput").ap()
    w_in = nc.dram_tensor("w_in", [P, DT, 2 * D], F16, kind="ExternalInput").ap()
    w_out = nc.dram_tensor("w_out", [P, ET, D], F16, kind="ExternalInput").ap()
    nu = nc.dram_tensor("nu", [P, DT], F16, kind="ExternalInput").ap()
    out = nc.dram_tensor("out", [n_len, D], F32, kind="ExternalOutput").ap()

    with tile.TileContext(nc) as tc:
        with (
            tc.tile_pool(name="const", bufs=1) as cpool,
            tc.tile_pool(name="io", bufs=2) as io,
            tc.tile_pool(name="work", bufs=2) as work,
            tc.tile_pool(name="m", bufs=2) as mpool,
            tc.tile_pool(name="feat", bufs=2) as fpool,
            tc.tile_pool(name="ost", bufs=3) as opool,
            tc.tile_pool(name="psin", bufs=2, space="PSUM") as psin,
            tc.tile_pool(name="psout", bufs=2, space="PSUM") as psout,
        ):
            w_in_sb = cpool.tile([P, DT, 2 * D], F16)
            nc.sync.dma_start(w_in_sb[:], w_in[:])
            w_out_sb = cpool.tile([P, ET, D], F16)
            nc.sync.dma_start(w_out_sb[:], w_out[:])
            nu_sb = cpool.tile([P, DT], F16)
            nc.sync.dma_start(nu_sb[:], nu[:])

            prev_mr = [None] * DT
            prev_mi = [None] * DT

            def emit_out_proj(k, feat):
                for nt in range(chunk // P):
                    for dc in range(D // 512):
                        po = psout.tile([P, 512], F32, tag="po")
                        for et in range(ET):
                            nc.tensor.matmul(
                                po[:],
                                feat[:, et, nt * P : (nt + 1) * P],
                                w_out_sb[:, et, dc * 512 : (dc + 1) * 512],
                                start=(et == 0),
                                stop=(et == ET - 1),
                            )
                        ot = opool.tile([P, 512], F32, tag="ot")
                        nc.scalar.copy(ot[:], po[:])
                        nc.sync.dma_start(
                            out[
                                k * chunk + nt * P : k * chunk + (nt + 1) * P,
                                dc * 512 : (dc + 1) * 512,
                            ],
                            ot[:],
                        )

            pending = None
            for k in range(nchunks):
                x_sb = io.tile([P, DT, chunk], F16, tag="x")
                nc.sync.dma_start(x_sb[:], xT[k])
                c_sb = io.tile([P, DT, chunk], F16, tag="c")
                nc.sync.dma_start(c_sb[:], ct[k])
                s_sb = io.tile([P, DT, chunk], F16, tag="s")
                nc.sync.dma_start(s_sb[:], st[k])
                feat = fpool.tile([P, ET, chunk], F16, tag="feat")

                for r in range(DT):
                    ps_r = psin.tile([P, chunk], F32, tag="psr")
                    for kt in range(DT):
                        nc.tensor.matmul(
                            ps_r[:],
                            w_in_sb[:, kt, r * P : (r + 1) * P],
                            x_sb[:, kt, :],
                            start=(kt == 0),
                            stop=(kt == DT - 1),
                        )
                    ps_i = psin.tile([P, chunk], F32, tag="psi")
                    for kt in range(DT):
                        nc.tensor.matmul(
                            ps_i[:],
                            w_in_sb[:, kt, (DT + r) * P : (DT + r + 1) * P],
                            x_sb[:, kt, :],
                            start=(kt == 0),
                            stop=(kt == DT - 1),
                        )
                    sr = work.tile([P, chunk], F16, tag="sr")
                    nc.scalar.copy(sr[:], ps_r[:])
                    si = work.tile([P, chunk], F16, tag="si")
                    nc.scalar.copy(si[:], ps_i[:])

                    cc = c_sb[:, r, :]
                    ss = s_sb[:, r, :]
                    # u' = e^{-i theta t} * u  (rotate into the real-scan basis)
                    t1 = work.tile([P, chunk], F16, tag="t1")
                    nc.vector.tensor_mul(t1[:], sr[:], cc)
                    t2 = work.tile([P, chunk], F16, tag="t2")
                    nc.vector.tensor_mul(t2[:], si[:], ss)
                    xr = work.tile([P, chunk], F16, tag="xr")
                    nc.vector.tensor_add(xr[:], t1[:], t2[:])
                    t3 = work.tile([P, chunk], F16, tag="t1")
                    nc.vector.tensor_mul(t3[:], si[:], cc)
                    t4 = work.tile([P, chunk], F16, tag="t2")
                    nc.vector.tensor_mul(t4[:], sr[:], ss)
                    xi = work.tile([P, chunk], F16, tag="xi")
                    nc.vector.tensor_sub(xi[:], t3[:], t4[:])

                    mr = mpool.tile([P, chunk], F16, tag=f"mr{r}")
                    mi = mpool.tile([P, chunk], F16, tag=f"mi{r}")
                    nu_b = nu_sb[:, r : r + 1].to_broadcast((P, chunk))
                    init_r = 0.0 if k == 0 else prev_mr[r][:, chunk - 1 : chunk]
                    init_i = 0.0 if k == 0 else prev_mi[r][:, chunk - 1 : chunk]
                    nc.vector.tensor_tensor_scan(mr[:], nu_b, xr[:], init_r, MULT, ADD)
                    nc.vector.tensor_tensor_scan(mi[:], nu_b, xi[:], init_i, MULT, ADD)
                    prev_mr[r] = mr
                    prev_mi[r] = mi

                    # h = e^{+i theta t} * m  (rotate back), directly into feature
                    u1 = work.tile([P, chunk], F16, tag="u1")
                    nc.gpsimd.tensor_mul(u1[:], cc, mr[:])
                    u2 = work.tile([P, chunk], F16, tag="u2")
                    nc.gpsimd.tensor_mul(u2[:], ss, mi[:])
                    nc.gpsimd.tensor_sub(feat[:, r, :], u1[:], u2[:])
                    u3 = work.tile([P, chunk], F16, tag="u1")
                    nc.gpsimd.tensor_mul(u3[:], ss, mr[:])
                    u4 = work.tile([P, chunk], F16, tag="u2")
                    nc.gpsimd.tensor_mul(u4[:], cc, mi[:])
                    nc.gpsimd.tensor_add(feat[:, DT + r, :], u3[:], u4[:])

                if pending is not None:
                    emit_out_proj(k - 1, pending)
                pending = feat
            emit_out_proj(nchunks - 1, pending)

    nc.compile()
    return nc


def _to_dev_tiles(a, n_len, chunk):
    # (D, n_len) -> (nchunks, P, DT, chunk), with d = tile*128 + p
    nchunks = n_len // chunk
    return np.ascontiguousarray(
        a.reshape(DT, P, nchunks, chunk).transpose(2, 1, 0, 3)
    )


def prepare_host_inputs(x, nu_log, theta_log, gamma_log, w_in, b_in, w_out, b_out,
                        n_len: int = N, chunk: int = CHUNK):
    """Build the per-core device input maps plus host-side bias correction."""
    x = np.asarray(x, dtype=np.float32)
    nu = np.exp(np.asarray(nu_log, dtype=np.float64))
    theta = np.exp(np.asarray(theta_log, dtype=np.float64))
    gamma = np.exp(-np.exp(np.asarray(gamma_log, dtype=np.float64)))

    t = np.arange(n_len, dtype=np.float64)
    ang = theta[:, None] * t[None, :]          # (D, n)
    ct_dev = _to_dev_tiles(np.cos(ang), n_len, chunk).astype(np.float16)
    st_dev = _to_dev_tiles(np.sin(ang), n_len, chunk).astype(np.float16)

    w64 = np.asarray(w_in, dtype=np.float64)
    wp = np.concatenate(
        [gamma[:, None] * w64[0::2, :], gamma[:, None] * w64[1::2, :]], axis=0
    )                                           # (2D, D), rows = e' order
    w_in_dev = np.ascontiguousarray(
        wp.T.reshape(DT, P, 2 * D).transpose(1, 0, 2)
    ).astype(np.float16)
    w_out_dev = np.ascontiguousarray(
        np.asarray(w_out, dtype=np.float64).T.reshape(ET, P, D).transpose(1, 0, 2)
    ).astype(np.float16)
    nu_dev = np.ascontiguousarray(nu.reshape(DT, P).T).astype(np.float16)

    shared = {
        "ct": ct_dev, "st": st_dev, "w_in": w_in_dev, "w_out": w_out_dev,
        "nu": nu_dev,
    }
    in_maps = []
    nb = x.shape[1]
    for b in range(nb):
        xT = np.asarray(x[:, b, :], dtype=np.float64).T   # (D, n)
        x_dev = _to_dev_tiles(xT, n_len, chunk).astype(np.float16)
        in_maps.append({"xT": x_dev, **shared})

    # Exact host-side correction for the (linear) effect of b_in and b_out.
    b_in64 = np.asarray(b_in, dtype=np.float64)
    b_out64 = np.asarray(b_out, dtype=np.float64)
    corr = None
    if np.any(b_in64):
        beta = gamma * (b_in64[0::2] + 1j * b_in64[1::2])          # (D,)
        a = nu * np.exp(1j * theta)                                 # (D,)
        # h_corr[t, d] = beta_d * (1 - a_d^{t+1}) / (1 - a_d)
        pw = a[None, :] ** (t[:, None] + 1.0)                       # (n, D)
        h_corr = beta[None, :] * (1.0 - pw) / (1.0 - a[None, :])
        featc = np.concatenate([h_corr.real, h_corr.imag], axis=1)  # (n, 2D)
        corr = featc @ np.asarray(w_out, dtype=np.float64).T        # (n, D)
    if np.any(b_out64):
        corr = b_out64[None, :] if corr is None else corr + b_out64[None, :]
    return in_maps, corr


_PROGRAM = None


def _get_program():
    global _PROGRAM
    if _PROGRAM is None:
        _PROGRAM = build_program()
    return _PROGRAM


def _install_ntff_hook():
    """Wire up antenv.axon_hooks with the boot shim's ctypes NTFF hook so
    run_bass_kernel_spmd(trace=True) can capture hardware profiles."""
    import sys
    import types

    try:
        import antenv.axon_hooks  # noqa: F401
        return True
    except ImportError:
        pass
    try:
        from trn_agent_boot.trn_boot import _ntff_profile_via_ctypes

        hook = _ntff_profile_via_ctypes("/opt/axon/libaxon_pjrt.so")
        mod = types.ModuleType("antenv.axon_hooks")
        mod._hook = hook
        mod.get_axon_ntff_profile_hook = lambda: mod._hook
        mod.set_axon_ntff_profile_hook = lambda h: setattr(mod, "_hook", h)
        sys.modules["antenv.axon_hooks"] = mod
        import antenv

        antenv.axon_hooks = mod
        return hook is not None
    except Exception as e:  # pragma: no cover - profiling is best-effort
        print(f"NTFF hook install failed: {e}")
        return False


def run_device(in_maps, trace: bool = False):
    nc = _get_program()
    if trace:
        _install_ntff_hook()
    res = bass_utils.run_bass_kernel_spmd(
        nc, in_maps, core_ids=list(range(len(in_maps))), trace=trace
    )
    return res


def kernel(x, nu_log, theta_log, gamma_log, w_in, b_in, w_out, b_out):
    in_maps, corr = prepare_host_inputs(
        x, nu_log, theta_log, gamma_log, w_in, b_in, w_out, b_out
    )
    res = run_device(in_maps)
    y = np.stack([res.results[b]["out"] for b in range(len(in_maps))], axis=1)
    if corr is not None:
        y = (y.astype(np.float64) + corr[:, None, :]).astype(np.float32)
    return y.astype(np.float32)
